# revision 2
# baseline (speedup 1.0000x reference)
"""Fused tensor-parallel MultiHeadAttention (GQA + RMSNorm-KV + RoPE), v2.

Per-core: 1 KV head, 2 Q heads; x replicated; Wo row-sharded; host sums
partial outputs (scaled by 2^-16).

Pipeline: for each 512-token block: fp8-DoubleRow projections (3-term
hi/lo error-compensated split) -> fp8 attention (fine-grained causal)
-> fp8 output projection.  All phases interleave on the PE; no DRAM
round-trips for q/k/v.

Scales: x/32 -> fp8, W/1024 -> fp8 (psum = value * 2^15);
q,k -> fp8 at /16; P = exp(s/16 - 2) at scale 1; V-hat at /16 (folded
with 16 into vsc); O at /64 via rb = 0.25*sum(P); y_psum = y * 2^16,
host multiplies the summed output by 2^-16.
"""
import sys
sys.path.insert(0, '/opt/trn_rl_repo')
import numpy as np
import concourse.bass as bass
import concourse.tile as tile
from concourse import mybir
from contextlib import ExitStack

F32 = mybir.dt.float32
F32R = mybir.dt.float32r
BF16 = mybir.dt.bfloat16
FP8 = mybir.dt.float8e4
DR = mybir.MatmulPerfMode.DoubleRow
AF = mybir.ActivationFunctionType
ALU = mybir.AluOpType

B = 2
S = 2048
D = 4096
HD = 256
ROPE_BASE = 10000.0
EPS = 1e-6
N_CORES = 8

# scales
S_X = 1.0 / 32
S_W = 1.0 / 1024
S_XW = S_X * S_W            # 2^-15
S_Q = 1.0 / 16
S_K = 1.0 / 16
S_V = 1.0 / 16
S_O = 1.0 / 64
EXP_SHIFT = 3.0
MASKV = -1e13
Y_SCALE = S_Q * S_W         # 2^-14: y_psum = y * 16 * 1024


def legalize_waits(nc, max_waits=1):
    """Walrus encodes at most one sem-wait per instruction; move extras
    onto same-engine NOPs just before (engine FIFO order equivalent)."""
    n = 0
    for f in nc.m.functions:
        for blk in f.blocks:
            out = []
            for ins in blk.instructions:
                si = ins.sync_info
                if si is not None and si.on_wait and len(si.on_wait) > max_waits:
                    waits = list(si.on_wait)
                    for w in waits[max_waits:]:
                        nop = mybir.InstNoOp(name=nc.get_next_instruction_name())
                        nop.engine = ins.engine
                        nop.sync_info = mybir.SyncInfo(on_wait=[w], on_update=[])
                        out.append(nop)
                    ins.sync_info = mybir.SyncInfo(
                        on_wait=waits[:max_waits], on_update=list(si.on_update or []))
                    n += 1
                out.append(ins)
            blk.instructions.clear()
            for i in out:
                blk.instructions.append(i)
    return n


def build_bass(b=B, s=S, d=D, legalize=True):
    T = b * s
    NB = T // 512          # 512-token blocks (4 per batch)
    BPB = s // 512         # blocks per batch
    NF = d // 256          # 256-deep contraction pairs
    NVG = s // 256         # v pair-groups per batch

    nc = bass.Bass()
    xh_d = nc.dram_tensor("xh", [d, T], FP8, kind="ExternalInput")
    xl_d = nc.dram_tensor("xl", [d, T], FP8, kind="ExternalInput")
    wqh_d = nc.dram_tensor("wqh", [d, 512], FP8, kind="ExternalInput")
    wql_d = nc.dram_tensor("wql", [d, 512], FP8, kind="ExternalInput")
    wkvh_d = nc.dram_tensor("wkvh", [d, 512], FP8, kind="ExternalInput")
    wkvl_d = nc.dram_tensor("wkvl", [d, 512], FP8, kind="ExternalInput")
    woh_d = nc.dram_tensor("woh", [512, d], FP8, kind="ExternalInput")
    wol_d = nc.dram_tensor("wol", [512, d], FP8, kind="ExternalInput")
    cosq_d = nc.dram_tensor("cosq", [128, T], BF16, kind="ExternalInput")
    sinq_d = nc.dram_tensor("sinq", [128, T], BF16, kind="ExternalInput")
    ksc_d = nc.dram_tensor("ksc", [128, 2], F32, kind="ExternalInput")
    vsc_d = nc.dram_tensor("vsc", [128, 256], BF16, kind="ExternalInput")
    tri_d = nc.dram_tensor("tri", [128, 128], BF16, kind="ExternalInput")
    y = nc.dram_tensor("y", [T, d], BF16, kind="ExternalOutput")

    with tile.TileContext(nc) as tc, ExitStack() as top:
        top.enter_context(
            nc.allow_low_precision(reason="fp8/bf16 staged kernel"))
        cp = top.enter_context(tc.tile_pool(name="const", bufs=1))
        zero_b = cp.tile([128, 1], F32)
        nc.vector.memset(zero_b[:], 0.0)
        eps_b = cp.tile([128, 1], F32)
        nc.vector.memset(eps_b[:], EPS)
        nexp_b = cp.tile([128, 1], F32)
        nc.vector.memset(nexp_b[:], -EXP_SHIFT)
        ones_bf = cp.tile([128, 128], BF16)
        nc.vector.memset(ones_bf[:], 1.0)
        ksc_sb = cp.tile([128, 2], F32)
        nc.sync.dma_start(ksc_sb[:], ksc_d[:])
        vsc_sb = cp.tile([128, 256], BF16)
        nc.sync.dma_start(vsc_sb[:], vsc_d[:])
        tri = cp.tile([128, 128], BF16)
        nc.sync.dma_start(tri[:], tri_d[:])

        wp = top.enter_context(tc.tile_pool(name="wpool", bufs=1))
        wqh = wp.tile([128, NF, 2, 512], FP8)
        wql = wp.tile([128, NF, 2, 512], FP8)
        wkvh = wp.tile([128, NF, 2, 512], FP8)
        wkvl = wp.tile([128, NF, 2, 512], FP8)
        woh = wp.tile([128, 2, 2, d], FP8)
        wol = wp.tile([128, 2, 2, d], FP8)

        def w_split(dst, src, spv, nsplit=8):
            f0, f1 = spv * NF // nsplit, (spv + 1) * NF // nsplit
            nc.sync.dma_start(
                dst[:, f0:f1, :, :],
                src[f0 * 256:f1 * 256, :].rearrange(
                    "(f i p) c -> p f i c", p=128, i=2))

        xp = top.enter_context(tc.tile_pool(name="xpool", bufs=1))
        cq = top.enter_context(tc.tile_pool(name="cqpool", bufs=2))
        qp = top.enter_context(tc.tile_pool(name="qpool", bufs=2))
        kvp = top.enter_context(tc.tile_pool(name="kvpool", bufs=1))
        stp = top.enter_context(tc.tile_pool(name="stage", bufs=1))
        pp = top.enter_context(tc.tile_pool(name="ppool", bufs=2))
        yp = top.enter_context(tc.tile_pool(name="ypool", bufs=2))
        otp = top.enter_context(tc.tile_pool(name="otpool", bufs=2))

        accp = top.enter_context(tc.tile_pool(name="accps", bufs=3, space="PSUM"))
        sp_ps = top.enter_context(tc.tile_pool(name="sps", bufs=2, space="PSUM"))
        op_ps = top.enter_context(tc.tile_pool(name="ops", bufs=1, space="PSUM"))
        rp_ps = top.enter_context(tc.tile_pool(name="rps", bufs=1, space="PSUM"))

        kTh = kvp.tile([128, 2, s], FP8)
        kTl = kvp.tile([128, 2, s], FP8)
        vbf = kvp.tile([128, NVG, 2, 256], BF16)

        def x_dma(blk, nchunk=1):
            xhb = xp.tile([128, NF, 2, 512], FP8, tag="xh", bufs=2)
            xlb = xp.tile([128, NF, 2, 512], FP8, tag="xl", bufs=1)
            c0 = blk * 512
            for cc in range(nchunk):
                f0, f1 = cc * NF // nchunk, (cc + 1) * NF // nchunk
                nc.sync.dma_start(
                    xhb[:, f0:f1, :, :],
                    xh_d[f0 * 256:f1 * 256, c0:c0 + 512].rearrange(
                        "(f i p) t -> p f i t", p=128, i=2))
            for cc in range(nchunk):
                f0, f1 = cc * NF // nchunk, (cc + 1) * NF // nchunk
                nc.sync.dma_start(
                    xlb[:, f0:f1, :, :],
                    xl_d[f0 * 256:f1 * 256, c0:c0 + 512].rearrange(
                        "(f i p) t -> p f i t", p=128, i=2))
            return xhb, xlb

        def cos_dma(blk):
            c0 = blk * 512
            cosb = cq.tile([128, 512], BF16, tag="cos")
            nc.sync.dma_start(cosb[:], cosq_d[:, c0:c0 + 512])
            sinb = cq.tile([128, 512], BF16, tag="sin")
            nc.sync.dma_start(sinb[:], sinq_d[:, c0:c0 + 512])
            return cosb, sinb

        # staged prologue: interleave weight f-slices with x block0 chunks
        # so the first Q tile's f-major consumption streams smoothly.
        for spv in range(4):
            w_split(wqh, wqh_d, spv, nsplit=4)
            w_split(wql, wql_d, spv, nsplit=4)
        xhb, xlb = x_dma(0, nchunk=4)
        cosb, sinb = cos_dma(0)
        for spv in range(4):
            w_split(wkvh, wkvh_d, spv, nsplit=4)
            w_split(wkvl, wkvl_d, spv, nsplit=4)
        x_next = x_dma(1)
        cos_next = cos_dma(1)
        for g in range(4):
            nc.sync.dma_start(
                woh[:, g // 2, g % 2, :], woh_d[g * 128:(g + 1) * 128, :])
        for g in range(4):
            nc.sync.dma_start(
                wol[:, g // 2, g % 2, :], wol_d[g * 128:(g + 1) * 128, :])

        def acc3(term, xt, wt):
            # 3-term hi/lo split: (xh,wh), (xh,wl), (xl,wh)
            if term == 0:
                return xt[0], wt[0]
            if term == 1:
                return xt[0], wt[1]
            return xt[1], wt[0]

        def emit_A(blk, xhb, xlb, cosb, sinb, drip):
            """Projections + norm + rope for one 512-token block.
            `drip` is a list of deferred y-tile thunks from the previous
            block's output projection, emitted at safe seams so their PE
            work interleaves with this block's accumulations."""
            c0 = (blk % BPB) * 512   # within-batch token offset
            xt = (xhb, xlb)

            def seam(k=3):
                for _ in range(min(k, len(drip))):
                    drip.pop(0)()

            qh = qp.tile([128, 2, 2, 512], FP8, tag="qh")
            ql = qp.tile([128, 2, 2, 512], FP8, tag="ql")

            # ---- Q^T direct: per (head, i) accumulate [128, 512] ----
            for h in range(2):
                qps = []
                for i in range(2):
                    ps = accp.tile([128, 512], F32, tag="acc")
                    m0 = (h * 2 + i) * 128
                    for t3, f in [(a, b) for b in range(NF)
                                  for a in (0, 1)] + [(2, b)
                                                      for b in range(NF)]:
                        xa, wa = acc3(t3, xt, (wqh, wql))
                        nc.tensor.matmul(
                            ps[:], wa[:, f, :, m0:m0 + 128],
                            xa[:, f, :, :],
                            start=(f == 0 and t3 == 0),
                            stop=(f == NF - 1 and t3 == 2),
                            perf_mode=DR)
                    qps.append(ps)
                # rope on (i0, i1) psum pair -> bf16 staging, then split
                qs = stp.tile([128, 2, 512], BF16, tag="qs", bufs=1)
                t1 = stp.tile([128, 512], BF16, tag="t1")
                t2 = stp.tile([128, 512], BF16, tag="t2")
                nc.vector.tensor_mul(t1[:], qps[0][:], cosb[:])
                nc.vector.tensor_mul(t2[:], qps[1][:], sinb[:])
                nc.vector.tensor_sub(qs[:, 0, :], t1[:], t2[:])
                nc.vector.tensor_mul(t1[:], qps[1][:], cosb[:])
                nc.vector.tensor_mul(t2[:], qps[0][:], sinb[:])
                nc.vector.tensor_add(qs[:, 1, :], t1[:], t2[:])
                qsc = S_XW / S_Q
                nc.vector.tensor_scalar_mul(qh[:, :, h, :], qs[:], qsc)
                nc.vector.scalar_tensor_tensor(
                    ql[:, :, h, :], qs[:], qsc, qh[:, :, h, :],
                    op0=ALU.mult, op1=ALU.subtract)
                seam()

            # ---- K^T direct + rmsnorm + rope ----
            k2 = stp.tile([128, 2, 512], BF16, tag="k2", bufs=1)
            kn = stp.tile([128, 2, 512], BF16, tag="kn", bufs=1)
            for i in range(2):
                ps = accp.tile([128, 512], F32, tag="acc")
                m0 = i * 128
                for t3, f in [(a, b) for b in range(NF)
                              for a in (0, 1)] + [(2, b)
                                                  for b in range(NF)]:
                    xa, wa = acc3(t3, xt, (wkvh, wkvl))
                    nc.tensor.matmul(
                        ps[:], wa[:, f, :, m0:m0 + 128],
                        xa[:, f, :, :],
                        start=(f == 0 and t3 == 0),
                        stop=(f == NF - 1 and t3 == 2),
                        perf_mode=DR)
                nc.scalar.activation(k2[:, i, :], ps[:], AF.Square,
                                     bias=zero_b[:])
                nc.vector.tensor_scalar_mul(kn[:, i, :], ps[:],
                                            ksc_sb[:, i:i + 1])
            # ssq via matmul (partition reduction), shares the s-pool ring
            ssq = sp_ps.tile([128, 512], F32, tag="s")
            nc.tensor.matmul(ssq[:], ones_bf[:], k2[:, 0, :],
                             start=True, stop=False)
            nc.tensor.matmul(ssq[:], ones_bf[:], k2[:, 1, :],
                             start=False, stop=True)
            stdt = stp.tile([128, 512], BF16, tag="stdt")
            nc.scalar.activation(stdt[:], ssq[:], AF.Sqrt,
                                 bias=eps_b[:], scale=S_XW * S_XW / 256.0)
            rstd = stp.tile([128, 512], BF16, tag="rstd")
            nc.vector.reciprocal(rstd[:], stdt[:])
            nc.vector.tensor_mul(kn[:, 0, :], kn[:, 0, :], rstd[:])
            nc.vector.tensor_mul(kn[:, 1, :], kn[:, 1, :], rstd[:])
            # rope
            kr = stp.tile([128, 2, 512], BF16, tag="kr", bufs=1)
            t1 = stp.tile([128, 512], BF16, tag="t1")
            t2 = stp.tile([128, 512], BF16, tag="t2")
            nc.vector.tensor_mul(t1[:], kn[:, 0, :], cosb[:])
            nc.vector.tensor_mul(t2[:], kn[:, 1, :], sinb[:])
            nc.vector.tensor_sub(kr[:, 0, :], t1[:], t2[:])
            nc.vector.tensor_mul(t1[:], kn[:, 1, :], cosb[:])
            nc.vector.tensor_mul(t2[:], kn[:, 0, :], sinb[:])
            nc.vector.tensor_add(kr[:, 1, :], t1[:], t2[:])
            nc.scalar.copy(kTh[:, :, c0:c0 + 512], kr[:])
            nc.vector.tensor_sub(kTl[:, :, c0:c0 + 512], kr[:],
                                 kTh[:, :, c0:c0 + 512])
            seam()

            # ---- V (token-major) + rmsnorm ----
            for c in range(4):
                ps = accp.tile([128, 256], F32, tag="acc")
                t0 = c * 128
                for t3, f in [(a, b) for b in range(NF)
                              for a in (0, 1)] + [(2, b)
                                                  for b in range(NF)]:
                    xa, wa = acc3(t3, xt, (wkvh, wkvl))
                    nc.tensor.matmul(
                        ps[:], xa[:, f, :, t0:t0 + 128],
                        wa[:, f, :, 256:512],
                        start=(f == 0 and t3 == 0),
                        stop=(f == NF - 1 and t3 == 2),
                        perf_mode=DR)
                sqv = stp.tile([128, 512], BF16, tag="t1")
                ssqv = stp.tile([128, 1], F32, tag="ssqv")
                nc.scalar.activation(sqv[:, 0:256], ps[:], AF.Square,
                                     bias=zero_b[:], accum_out=ssqv[:])
                stdv = stp.tile([128, 1], F32, tag="stdv")
                nc.scalar.activation(stdv[:], ssqv[:], AF.Sqrt,
                                     bias=eps_b[:],
                                     scale=S_XW * S_XW / 256.0)
                rstdv = stp.tile([128, 1], F32, tag="rstdv")
                nc.vector.reciprocal(rstdv[:], stdv[:])
                gi = (blk % BPB) * 4 + c
                g, ii = gi // 2, gi % 2
                nc.vector.scalar_tensor_tensor(
                    vbf[:, g, ii, :], ps[:], rstdv[:], vsc_sb[:],
                    op0=ALU.mult, op1=ALU.mult)
                seam()
            seam(99)
            return qh, ql

        def emit_attn(blk, qh, ql):
            """Attention for one 512-query block (both heads).

            Scores use fp8 DoubleRow 3-term hi/lo; P and V stay bf16 so
            softmax weights never hit the fp8 subnormal cutoff.  PV and
            rowsum matmuls for a key block are deferred behind later
            blocks' score emission so the exp latency is hidden from the
            in-order PE queue."""
            kb = blk % BPB
            nj = 4 * kb + 4
            oth = otp.tile([128, 2, 2, 512], FP8, tag="oth")
            otl = otp.tile([128, 2, 2, 512], FP8, tag="otl")
            for h in range(2):
                o0 = op_ps.tile([128, 512], F32, tag="o0")
                o1 = op_ps.tile([128, 512], F32, tag="o1")
                rb = rp_ps.tile([128, 512], F32, tag="rb")
                pending = []

                def emit_pv(ent):
                    j, off, g, jj, pbf = ent
                    nc.tensor.matmul(
                        rb[:, off:512], ones_bf[:], pbf[:, off:512],
                        start=(j == 0), stop=(j == nj - 1),
                        skip_group_check=True)
                    nc.tensor.matmul(
                        o0[:, off:512], vbf[:, g, jj, 0:128],
                        pbf[:, off:512],
                        start=(j == 0), stop=(j == nj - 1),
                        skip_group_check=True)
                    nc.tensor.matmul(
                        o1[:, off:512], vbf[:, g, jj, 128:256],
                        pbf[:, off:512],
                        start=(j == 0), stop=(j == nj - 1),
                        skip_group_check=True)

                for j in range(nj):
                    off = max(0, j * 128 - kb * 512)
                    s_ps = sp_ps.tile([128, 512], F32, tag="s")
                    for t3 in range(3):
                        ka = (kTh, kTh, kTl)[t3]
                        qa = (qh, ql, qh)[t3]
                        nc.tensor.matmul(
                            s_ps[:, off:512],
                            ka[:, :, j * 128:(j + 1) * 128],
                            qa[:, :, h, off:512],
                            start=(t3 == 0), stop=(t3 == 2),
                            perf_mode=DR)
                    if off > 0 or j * 128 == kb * 512:
                        nc.vector.tensor_add(
                            s_ps[:, off:off + 128],
                            s_ps[:, off:off + 128], tri[:])
                    pbf = pp.tile([128, 512], BF16, tag="pbf", bufs=4)
                    nc.scalar.activation(
                        pbf[:, off:512], s_ps[:, off:512], AF.Exp,
                        bias=nexp_b[:], scale=S_Q * S_K / 16.0)
                    pending.append((j, off, j // 2, j % 2, pbf))
                    if len(pending) > 2:
                        emit_pv(pending.pop(0))
                for ent in pending:
                    emit_pv(ent)
                # normalize + split O
                recip = stp.tile([128, 512], BF16, tag="recip")
                nc.vector.reciprocal(recip[:], rb[:])
                for i, ops_ in enumerate((o0, o1)):
                    tmp = stp.tile([128, 512], BF16, tag="otmp", bufs=1)
                    nc.vector.tensor_mul(tmp[:], ops_[:], recip[:])
                    nc.scalar.copy(oth[:, h, i, :], tmp[:])
                    nc.vector.tensor_sub(otl[:, h, i, :], tmp[:],
                                         oth[:, h, i, :])
            return oth, otl

        def emit_D_tiles(blk, oth, otl):
            """Output projection for one block, as deferred y-tile thunks."""
            ot = (oth, otl)
            wt = (woh, wol)

            def mk(tt, eb):
                def thunk():
                    t0 = tt * 128
                    yps = accp.tile([128, 512], F32, tag="acc")
                    n = 0
                    for e in range(2):
                        for t3 in range(3):
                            oa = ot[(0, 0, 1)[t3]]
                            wa = wt[(0, 1, 0)[t3]]
                            nc.tensor.matmul(
                                yps[:], oa[:, e, :, t0:t0 + 128],
                                wa[:, e, :, eb * 512:(eb + 1) * 512],
                                start=(n == 0), stop=(n == 5),
                                perf_mode=DR)
                            n += 1
                    ysb = yp.tile([128, 512], BF16, tag="y")
                    r0 = blk * 512 + t0
                    if (tt * 8 + eb) % 2 == 0:
                        nc.scalar.copy(ysb[:], yps[:])
                        nc.scalar.dma_start(
                            y[r0:r0 + 128, eb * 512:(eb + 1) * 512], ysb[:])
                    else:
                        nc.vector.tensor_copy(ysb[:], yps[:])
                        nc.scalar.dma_start(
                            y[r0:r0 + 128, eb * 512:(eb + 1) * 512], ysb[:])
                return thunk
            return [mk(tt, eb) for tt in range(4) for eb in range(d // 512)]

        drip = []
        for blk in range(NB):
            qh, ql = emit_A(blk, xhb, xlb, cosb, sinb, drip)
            if blk + 1 < NB:
                xhb, xlb = x_next
                cosb, sinb = cos_next
            if blk + 2 < NB:
                x_next = x_dma(blk + 2, nchunk=4)
                cos_next = cos_dma(blk + 2)
            oth, otl = emit_attn(blk, qh, ql)
            drip = emit_D_tiles(blk, oth, otl)
        for t in drip:
            t()

    if legalize:
        legalize_waits(nc)
    return nc


def _fp8_split(a, scale):
    np8 = mybir.dt.np(FP8)
    hi = (a / scale).astype(np8)
    lo = (a / scale - hi.astype(np.float32)).astype(np8)
    return hi, lo


def host_common(x, position, b=B, s=S, d=D):
    T = b * s
    xT = np.ascontiguousarray(x.reshape(T, d).T).astype(np.float32)
    xh, xl = _fp8_split(xT, S_X)

    pos = position.reshape(T).astype(np.float32)
    j = np.arange(128, dtype=np.float32)
    timescale = ROPE_BASE ** (2.0 * j / HD)
    ang = pos[None, :] / timescale[:, None]        # [128, T]
    cosq = np.cos(ang).astype(mybir.dt.np(BF16))
    sinq = np.sin(ang).astype(mybir.dt.np(BF16))

    p = np.arange(128)[:, None]
    c = np.arange(128)[None, :]
    trim = np.where(p <= c, 0.0, MASKV).astype(mybir.dt.np(BF16))
    return {"xh": xh, "xl": xl, "cosq": cosq, "sinq": sinq, "tri": trim}


def host_inputs(common, Wq, Wk, Wv, Wo, k_scale, v_scale, core):
    wq_c = Wq[:, core * 512:(core + 1) * 512].astype(np.float32)
    wk_c = Wk[:, core * 256:(core + 1) * 256].astype(np.float32)
    wv_c = Wv[:, core * 256:(core + 1) * 256].astype(np.float32)
    wkv_c = np.concatenate([wk_c, wv_c], axis=1)
    wo_c = Wo[core * 512:(core + 1) * 512, :].astype(np.float32)
    wqh, wql = _fp8_split(wq_c, S_W)
    wkvh, wkvl = _fp8_split(wkv_c, S_W)
    woh, wol = _fp8_split(wo_c, S_W)

    ksc = np.empty((128, 2), dtype=np.float32)
    ksc[:, 0] = (1.0 + k_scale[0:128]) * (1.0 / S_K) * S_XW
    ksc[:, 1] = (1.0 + k_scale[128:256]) * (1.0 / S_K) * S_XW
    vsc = np.broadcast_to(
        ((1.0 + v_scale) * (1.0 / S_V) * S_XW).astype(mybir.dt.np(BF16)),
        (128, 256)).copy()

    out = {"wqh": wqh, "wql": wql, "wkvh": wkvh, "wkvl": wkvl,
           "woh": woh, "wol": wol, "ksc": ksc, "vsc": vsc}
    out.update(common)
    return out


def kernel(x, Wq, Wk, Wv, Wo, k_scale, v_scale, mask, position):
    from concourse.bass_utils import run_bass_kernel_spmd
    b, s, d = x.shape
    nc = build_bass(b=b, s=s, d=d)
    common = host_common(x, position, b=b, s=s, d=d)
    in_maps = [
        host_inputs(common, Wq, Wk, Wv, Wo, k_scale, v_scale, core)
        for core in range(N_CORES)
    ]
    res = run_bass_kernel_spmd(nc, in_maps, list(range(N_CORES)))
    out = None
    for r in res.results:
        yc = r["y"].astype(np.float32)
        out = yc if out is None else out + yc
    return (out * Y_SCALE).reshape(b, s, d).astype(np.float32)


# revision 5
# speedup vs baseline: 1.0146x; 1.0146x over previous
"""Fused tensor-parallel MultiHeadAttention (GQA + RMSNorm-KV + RoPE), v2.

Per-core: 1 KV head, 2 Q heads; x replicated; Wo row-sharded; host sums
partial outputs (scaled by 2^-16).

Pipeline: for each 512-token block: fp8-DoubleRow projections (3-term
hi/lo error-compensated split) -> fp8 attention (fine-grained causal)
-> fp8 output projection.  All phases interleave on the PE; no DRAM
round-trips for q/k/v.

Scales: x/32 -> fp8 hi/lo, W/1024 -> fp8 hi/lo (psum = value * 2^15);
q,k -> fp8 hi/lo at /16; P = exp(s/16 - 3) and V-hat kept in bf16 (no
fp8 subnormal cutoff in the softmax path); O -> fp8 hi/lo at /16 via
rb = sum(P); y_psum = y * 2^14, host multiplies the summed output by
Y_SCALE = 2^-14.
"""
import sys
sys.path.insert(0, '/opt/trn_rl_repo')
import numpy as np
import concourse.bass as bass
import concourse.tile as tile
from concourse import mybir
from contextlib import ExitStack

F32 = mybir.dt.float32
F32R = mybir.dt.float32r
BF16 = mybir.dt.bfloat16
FP8 = mybir.dt.float8e4
DR = mybir.MatmulPerfMode.DoubleRow
AF = mybir.ActivationFunctionType
ALU = mybir.AluOpType

B = 2
S = 2048
D = 4096
HD = 256
ROPE_BASE = 10000.0
EPS = 1e-6
N_CORES = 8

# scales
S_X = 1.0 / 32
S_W = 1.0 / 1024
S_XW = S_X * S_W            # 2^-15
S_Q = 1.0 / 16
S_K = 1.0 / 16
S_V = 1.0 / 16
S_O = 1.0 / 64
EXP_SHIFT = 3.0
MASKV = -1e13
Y_SCALE = S_Q * S_W         # 2^-14: y_psum = y * 16 * 1024


def legalize_waits(nc, max_waits=1):
    """Walrus encodes at most one sem-wait per instruction; move extras
    onto same-engine NOPs just before (engine FIFO order equivalent)."""
    n = 0
    for f in nc.m.functions:
        for blk in f.blocks:
            out = []
            for ins in blk.instructions:
                si = ins.sync_info
                if si is not None and si.on_wait and len(si.on_wait) > max_waits:
                    waits = list(si.on_wait)
                    for w in waits[max_waits:]:
                        nop = mybir.InstNoOp(name=nc.get_next_instruction_name())
                        nop.engine = ins.engine
                        nop.sync_info = mybir.SyncInfo(on_wait=[w], on_update=[])
                        out.append(nop)
                    ins.sync_info = mybir.SyncInfo(
                        on_wait=waits[:max_waits], on_update=list(si.on_update or []))
                    n += 1
                out.append(ins)
            blk.instructions.clear()
            for i in out:
                blk.instructions.append(i)
    return n


def build_bass(b=B, s=S, d=D, legalize=True):
    T = b * s
    NB = T // 512          # 512-token blocks (4 per batch)
    BPB = s // 512         # blocks per batch
    NF = d // 256          # 256-deep contraction pairs
    NVG = s // 256         # v pair-groups per batch

    nc = bass.Bass()
    xh_d = nc.dram_tensor("xh", [d, T], FP8, kind="ExternalInput")
    xl_d = nc.dram_tensor("xl", [d, T], FP8, kind="ExternalInput")
    wqh_d = nc.dram_tensor("wqh", [d, 512], FP8, kind="ExternalInput")
    wql_d = nc.dram_tensor("wql", [d, 512], FP8, kind="ExternalInput")
    wkvh_d = nc.dram_tensor("wkvh", [d, 512], FP8, kind="ExternalInput")
    wkvl_d = nc.dram_tensor("wkvl", [d, 512], FP8, kind="ExternalInput")
    woh_d = nc.dram_tensor("woh", [512, d], FP8, kind="ExternalInput")
    wol_d = nc.dram_tensor("wol", [512, d], FP8, kind="ExternalInput")
    cosq_d = nc.dram_tensor("cosq", [128, T], BF16, kind="ExternalInput")
    sinq_d = nc.dram_tensor("sinq", [128, T], BF16, kind="ExternalInput")
    ksc_d = nc.dram_tensor("ksc", [128, 2], F32, kind="ExternalInput")
    vsc_d = nc.dram_tensor("vsc", [128, 256], BF16, kind="ExternalInput")
    tri_d = nc.dram_tensor("tri", [128, 128], BF16, kind="ExternalInput")
    y = nc.dram_tensor("y", [T, d], BF16, kind="ExternalOutput")

    with tile.TileContext(nc) as tc, ExitStack() as top:
        top.enter_context(
            nc.allow_low_precision(reason="fp8/bf16 staged kernel"))
        cp = top.enter_context(tc.tile_pool(name="const", bufs=1))
        zero_b = cp.tile([128, 1], F32)
        nc.vector.memset(zero_b[:], 0.0)
        eps_b = cp.tile([128, 1], F32)
        nc.vector.memset(eps_b[:], EPS)
        nexp_b = cp.tile([128, 1], F32)
        nc.vector.memset(nexp_b[:], -EXP_SHIFT)
        ones_bf = cp.tile([128, 128], BF16)
        nc.vector.memset(ones_bf[:], 1.0)
        ksc_sb = cp.tile([128, 2], F32)
        nc.sync.dma_start(ksc_sb[:], ksc_d[:])
        vsc_sb = cp.tile([128, 256], BF16)
        nc.sync.dma_start(vsc_sb[:], vsc_d[:])
        tri = cp.tile([128, 128], BF16)
        nc.sync.dma_start(tri[:], tri_d[:])

        wp = top.enter_context(tc.tile_pool(name="wpool", bufs=1))
        wqh = wp.tile([128, NF, 2, 512], FP8)
        wql = wp.tile([128, NF, 2, 512], FP8)
        wkvh = wp.tile([128, NF, 2, 512], FP8)
        wkvl = wp.tile([128, NF, 2, 512], FP8)
        woh = wp.tile([128, 2, 2, d], FP8)
        wol = wp.tile([128, 2, 2, d], FP8)

        def w_split(dst, src, spv, nsplit=8):
            f0, f1 = spv * NF // nsplit, (spv + 1) * NF // nsplit
            nc.sync.dma_start(
                dst[:, f0:f1, :, :],
                src[f0 * 256:f1 * 256, :].rearrange(
                    "(f i p) c -> p f i c", p=128, i=2))

        xp = top.enter_context(tc.tile_pool(name="xpool", bufs=1))
        cq = top.enter_context(tc.tile_pool(name="cqpool", bufs=2))
        qp = top.enter_context(tc.tile_pool(name="qpool", bufs=2))
        kvp = top.enter_context(tc.tile_pool(name="kvpool", bufs=1))
        stp = top.enter_context(tc.tile_pool(name="stage", bufs=1))
        pp = top.enter_context(tc.tile_pool(name="ppool", bufs=2))
        yp = top.enter_context(tc.tile_pool(name="ypool", bufs=4))
        otp = top.enter_context(tc.tile_pool(name="otpool", bufs=2))

        accp = top.enter_context(tc.tile_pool(name="accps", bufs=3, space="PSUM"))
        sp_ps = top.enter_context(tc.tile_pool(name="sps", bufs=2, space="PSUM"))
        op_ps = top.enter_context(tc.tile_pool(name="ops", bufs=1, space="PSUM"))
        rp_ps = top.enter_context(tc.tile_pool(name="rps", bufs=1, space="PSUM"))

        kTh = kvp.tile([128, 2, s], FP8)
        kTl = kvp.tile([128, 2, s], FP8)
        vbf = kvp.tile([128, NVG, 2, 256], BF16)

        def x_dma(blk, nchunk=1):
            xhb = xp.tile([128, NF, 2, 512], FP8, tag="xh", bufs=2)
            xlb = xp.tile([128, NF, 2, 512], FP8, tag="xl", bufs=1)
            c0 = blk * 512
            for cc in range(nchunk):
                f0, f1 = cc * NF // nchunk, (cc + 1) * NF // nchunk
                nc.sync.dma_start(
                    xhb[:, f0:f1, :, :],
                    xh_d[f0 * 256:f1 * 256, c0:c0 + 512].rearrange(
                        "(f i p) t -> p f i t", p=128, i=2))
            for cc in range(nchunk):
                f0, f1 = cc * NF // nchunk, (cc + 1) * NF // nchunk
                nc.sync.dma_start(
                    xlb[:, f0:f1, :, :],
                    xl_d[f0 * 256:f1 * 256, c0:c0 + 512].rearrange(
                        "(f i p) t -> p f i t", p=128, i=2))
            return xhb, xlb

        def cos_dma(blk):
            c0 = blk * 512
            cosb = cq.tile([128, 512], BF16, tag="cos")
            nc.sync.dma_start(cosb[:], cosq_d[:, c0:c0 + 512])
            sinb = cq.tile([128, 512], BF16, tag="sin")
            nc.sync.dma_start(sinb[:], sinq_d[:, c0:c0 + 512])
            return cosb, sinb

        # staged prologue: weight f-slices then x block0 chunks
        for spv in range(4):
            w_split(wqh, wqh_d, spv, nsplit=4)
            w_split(wql, wql_d, spv, nsplit=4)
        xhb, xlb = x_dma(0, nchunk=4)
        cosb, sinb = cos_dma(0)
        for spv in range(4):
            w_split(wkvh, wkvh_d, spv, nsplit=4)
            w_split(wkvl, wkvl_d, spv, nsplit=4)
        x_next = x_dma(1)
        cos_next = cos_dma(1)
        for g in range(4):
            nc.sync.dma_start(
                woh[:, g // 2, g % 2, :], woh_d[g * 128:(g + 1) * 128, :])
        for g in range(4):
            nc.sync.dma_start(
                wol[:, g // 2, g % 2, :], wol_d[g * 128:(g + 1) * 128, :])

        def acc3(term, xt, wt):
            # 3-term hi/lo split: (xh,wh), (xh,wl), (xl,wh)
            if term == 0:
                return xt[0], wt[0]
            if term == 1:
                return xt[0], wt[1]
            return xt[1], wt[0]

        def emit_A(blk, xhb, xlb, cosb, sinb, drip):
            """Projections + norm + rope for one 512-token block.
            `drip` is a list of deferred y-tile thunks from the previous
            block's output projection, emitted at safe seams so their PE
            work interleaves with this block's accumulations."""
            c0 = (blk % BPB) * 512   # within-batch token offset
            xt = (xhb, xlb)

            def seam(k=3):
                for _ in range(min(k, len(drip))):
                    drip.pop(0)()

            qh = qp.tile([128, 2, 2, 512], FP8, tag="qh")
            ql = qp.tile([128, 2, 2, 512], FP8, tag="ql")

            # ---- Q^T direct: per (head, i) accumulate [128, 512] ----
            for h in range(2):
                qps = []
                for i in range(2):
                    ps = accp.tile([128, 512], F32, tag="acc")
                    m0 = (h * 2 + i) * 128
                    for t3, f in [(a, b) for b in range(NF)
                                  for a in (0, 1)] + [(2, b)
                                                      for b in range(NF)]:
                        xa, wa = acc3(t3, xt, (wqh, wql))
                        nc.tensor.matmul(
                            ps[:], wa[:, f, :, m0:m0 + 128],
                            xa[:, f, :, :],
                            start=(f == 0 and t3 == 0),
                            stop=(f == NF - 1 and t3 == 2),
                            perf_mode=DR)
                    qps.append(ps)
                # rope on (i0, i1) psum pair -> bf16 staging, then split
                qs = stp.tile([128, 2, 512], BF16, tag="qs", bufs=1)
                t1 = stp.tile([128, 512], BF16, tag="t1")
                t2 = stp.tile([128, 512], BF16, tag="t2")
                nc.vector.tensor_mul(t1[:], qps[0][:], cosb[:])
                nc.vector.tensor_mul(t2[:], qps[1][:], sinb[:])
                nc.vector.tensor_sub(qs[:, 0, :], t1[:], t2[:])
                nc.vector.tensor_mul(t1[:], qps[1][:], cosb[:])
                nc.vector.tensor_mul(t2[:], qps[0][:], sinb[:])
                nc.vector.tensor_add(qs[:, 1, :], t1[:], t2[:])
                qsc = S_XW / S_Q
                nc.vector.tensor_scalar_mul(qh[:, :, h, :], qs[:], qsc)
                nc.vector.scalar_tensor_tensor(
                    ql[:, :, h, :], qs[:], qsc, qh[:, :, h, :],
                    op0=ALU.mult, op1=ALU.subtract)
                seam()
                seam()

            # ---- K^T direct + rmsnorm + rope ----
            k2 = stp.tile([128, 2, 512], BF16, tag="k2", bufs=1)
            kn = stp.tile([128, 2, 512], BF16, tag="kn", bufs=1)
            for i in range(2):
                ps = accp.tile([128, 512], F32, tag="acc")
                m0 = i * 128
                for t3, f in [(a, b) for b in range(NF)
                              for a in (0, 1)] + [(2, b)
                                                  for b in range(NF)]:
                    xa, wa = acc3(t3, xt, (wkvh, wkvl))
                    nc.tensor.matmul(
                        ps[:], wa[:, f, :, m0:m0 + 128],
                        xa[:, f, :, :],
                        start=(f == 0 and t3 == 0),
                        stop=(f == NF - 1 and t3 == 2),
                        perf_mode=DR)
                nc.scalar.activation(k2[:, i, :], ps[:], AF.Square,
                                     bias=zero_b[:])
                nc.vector.tensor_scalar_mul(kn[:, i, :], ps[:],
                                            ksc_sb[:, i:i + 1])
            # ssq via matmul (partition reduction), shares the s-pool ring
            ssq = sp_ps.tile([128, 512], F32, tag="s")
            nc.tensor.matmul(ssq[:], ones_bf[:], k2[:, 0, :],
                             start=True, stop=False)
            nc.tensor.matmul(ssq[:], ones_bf[:], k2[:, 1, :],
                             start=False, stop=True)
            stdt = stp.tile([128, 512], F32, tag="stdt")
            nc.scalar.activation(stdt[:], ssq[:], AF.Sqrt,
                                 bias=eps_b[:], scale=S_XW * S_XW / 256.0)
            rstd = stp.tile([128, 512], F32, tag="rstd")
            nc.vector.reciprocal(rstd[:], stdt[:])
            nc.vector.tensor_mul(kn[:, 0, :], kn[:, 0, :], rstd[:])
            nc.vector.tensor_mul(kn[:, 1, :], kn[:, 1, :], rstd[:])
            # rope
            kr = stp.tile([128, 2, 512], BF16, tag="kr", bufs=1)
            t1 = stp.tile([128, 512], BF16, tag="t1")
            t2 = stp.tile([128, 512], BF16, tag="t2")
            nc.vector.tensor_mul(t1[:], kn[:, 0, :], cosb[:])
            nc.vector.tensor_mul(t2[:], kn[:, 1, :], sinb[:])
            nc.vector.tensor_sub(kr[:, 0, :], t1[:], t2[:])
            nc.vector.tensor_mul(t1[:], kn[:, 1, :], cosb[:])
            nc.vector.tensor_mul(t2[:], kn[:, 0, :], sinb[:])
            nc.vector.tensor_add(kr[:, 1, :], t1[:], t2[:])
            nc.scalar.copy(kTh[:, :, c0:c0 + 512], kr[:])
            nc.vector.tensor_sub(kTl[:, :, c0:c0 + 512], kr[:],
                                 kTh[:, :, c0:c0 + 512])
            seam()
            seam()

            # ---- V (token-major) + rmsnorm ----
            for c in range(4):
                ps = accp.tile([128, 256], F32, tag="acc")
                t0 = c * 128
                for t3, f in [(a, b) for b in range(NF)
                              for a in (0, 1)] + [(2, b)
                                                  for b in range(NF)]:
                    xa, wa = acc3(t3, xt, (wkvh, wkvl))
                    nc.tensor.matmul(
                        ps[:], xa[:, f, :, t0:t0 + 128],
                        wa[:, f, :, 256:512],
                        start=(f == 0 and t3 == 0),
                        stop=(f == NF - 1 and t3 == 2),
                        perf_mode=DR)
                sqv = stp.tile([128, 512], BF16, tag="t1")
                ssqv = stp.tile([128, 1], F32, tag="ssqv")
                nc.scalar.activation(sqv[:, 0:256], ps[:], AF.Square,
                                     bias=zero_b[:], accum_out=ssqv[:])
                stdv = stp.tile([128, 1], F32, tag="stdv")
                nc.scalar.activation(stdv[:], ssqv[:], AF.Sqrt,
                                     bias=eps_b[:],
                                     scale=S_XW * S_XW / 256.0)
                rstdv = stp.tile([128, 1], F32, tag="rstdv")
                nc.vector.reciprocal(rstdv[:], stdv[:])
                gi = (blk % BPB) * 4 + c
                g, ii = gi // 2, gi % 2
                nc.vector.scalar_tensor_tensor(
                    vbf[:, g, ii, :], ps[:], rstdv[:], vsc_sb[:],
                    op0=ALU.mult, op1=ALU.mult)
                seam()
            seam(99)
            return qh, ql

        def emit_attn(blk, qh, ql):
            """Attention for one 512-query block (both heads).

            Scores use fp8 DoubleRow 3-term hi/lo; P and V stay bf16 so
            softmax weights never hit the fp8 subnormal cutoff.  PV and
            rowsum matmuls for a key block are deferred behind later
            blocks' score emission so the exp latency is hidden from the
            in-order PE queue."""
            kb = blk % BPB
            nj = 4 * kb + 4
            oth = otp.tile([128, 2, 2, 512], FP8, tag="oth")
            otl = otp.tile([128, 2, 2, 512], FP8, tag="otl")
            for h in range(2):
                o0 = op_ps.tile([128, 512], F32, tag="o0")
                o1 = op_ps.tile([128, 512], F32, tag="o1")
                rb = rp_ps.tile([128, 512], F32, tag="rb")
                pending = []

                def emit_pv(ent):
                    j, off, g, jj, pbf = ent
                    nc.tensor.matmul(
                        rb[:, off:512], ones_bf[:], pbf[:, off:512],
                        start=(j == 0), stop=(j == nj - 1),
                        skip_group_check=True)
                    nc.tensor.matmul(
                        o0[:, off:512], vbf[:, g, jj, 0:128],
                        pbf[:, off:512],
                        start=(j == 0), stop=(j == nj - 1),
                        skip_group_check=True)
                    nc.tensor.matmul(
                        o1[:, off:512], vbf[:, g, jj, 128:256],
                        pbf[:, off:512],
                        start=(j == 0), stop=(j == nj - 1),
                        skip_group_check=True)

                for j in range(nj):
                    off = max(0, j * 128 - kb * 512)
                    s_ps = sp_ps.tile([128, 512], F32, tag="s")
                    for t3 in range(3):
                        ka = (kTh, kTh, kTl)[t3]
                        qa = (qh, ql, qh)[t3]
                        nc.tensor.matmul(
                            s_ps[:, off:512],
                            ka[:, :, j * 128:(j + 1) * 128],
                            qa[:, :, h, off:512],
                            start=(t3 == 0), stop=(t3 == 2),
                            perf_mode=DR)
                    if off > 0 or j * 128 == kb * 512:
                        nc.vector.tensor_add(
                            s_ps[:, off:off + 128],
                            s_ps[:, off:off + 128], tri[:])
                    pbf = pp.tile([128, 512], BF16, tag="pbf", bufs=4)
                    nc.scalar.activation(
                        pbf[:, off:512], s_ps[:, off:512], AF.Exp,
                        bias=nexp_b[:], scale=S_Q * S_K / 16.0)
                    pending.append((j, off, j // 2, j % 2, pbf))
                    if len(pending) > 2:
                        emit_pv(pending.pop(0))
                for ent in pending:
                    emit_pv(ent)
                # normalize + split O
                recip = stp.tile([128, 512], BF16, tag="recip")
                nc.vector.reciprocal(recip[:], rb[:])
                for i, ops_ in enumerate((o0, o1)):
                    tmp = stp.tile([128, 512], BF16, tag="otmp", bufs=1)
                    nc.vector.tensor_mul(tmp[:], ops_[:], recip[:])
                    nc.scalar.copy(oth[:, h, i, :], tmp[:])
                    nc.vector.tensor_sub(otl[:, h, i, :], tmp[:],
                                         oth[:, h, i, :])
            return oth, otl

        def emit_D_tiles(blk, oth, otl):
            """Output projection for one block, as deferred y-tile thunks."""
            ot = (oth, otl)
            wt = (woh, wol)

            def mk(tt, eb):
                def thunk():
                    t0 = tt * 128
                    r = (tt * 8 + eb) % 3
                    if r == 0:
                        yps = op_ps.tile([128, 512], F32, tag="o0")
                    elif r == 1:
                        yps = op_ps.tile([128, 512], F32, tag="o1")
                    else:
                        yps = rp_ps.tile([128, 512], F32, tag="rb")
                    n = 0
                    for e in range(2):
                        for t3 in range(3):
                            oa = ot[(0, 0, 1)[t3]]
                            wa = wt[(0, 1, 0)[t3]]
                            nc.tensor.matmul(
                                yps[:], oa[:, e, :, t0:t0 + 128],
                                wa[:, e, :, eb * 512:(eb + 1) * 512],
                                start=(n == 0), stop=(n == 5),
                                perf_mode=DR)
                            n += 1
                    ysb = yp.tile([128, 512], BF16, tag="y")
                    r0 = blk * 512 + t0
                    if (tt * 8 + eb) % 2 == 0:
                        nc.scalar.copy(ysb[:], yps[:])
                    else:
                        nc.vector.tensor_copy(ysb[:], yps[:])
                    nc.sync.dma_start(
                        y[r0:r0 + 128, eb * 512:(eb + 1) * 512], ysb[:])
                return thunk
            return [mk(tt, eb) for tt in range(4) for eb in range(d // 512)]

        drip = []
        for blk in range(NB):
            qh, ql = emit_A(blk, xhb, xlb, cosb, sinb, drip)
            if blk + 1 < NB:
                xhb, xlb = x_next
                cosb, sinb = cos_next
            if blk + 2 < NB:
                x_next = x_dma(blk + 2, nchunk=4)
                cos_next = cos_dma(blk + 2)
            oth, otl = emit_attn(blk, qh, ql)
            drip = emit_D_tiles(blk, oth, otl)
        for t in drip:
            t()

    if legalize:
        legalize_waits(nc)
    return nc


def _fp8_split(a, scale):
    np8 = mybir.dt.np(FP8)
    hi = (a / scale).astype(np8)
    lo = (a / scale - hi.astype(np.float32)).astype(np8)
    return hi, lo


def host_common(x, position, b=B, s=S, d=D):
    T = b * s
    xT = np.ascontiguousarray(x.reshape(T, d).T).astype(np.float32)
    xh, xl = _fp8_split(xT, S_X)

    pos = position.reshape(T).astype(np.float32)
    j = np.arange(128, dtype=np.float32)
    timescale = ROPE_BASE ** (2.0 * j / HD)
    ang = pos[None, :] / timescale[:, None]        # [128, T]
    cosq = np.cos(ang).astype(mybir.dt.np(BF16))
    sinq = np.sin(ang).astype(mybir.dt.np(BF16))

    p = np.arange(128)[:, None]
    c = np.arange(128)[None, :]
    trim = np.where(p <= c, 0.0, MASKV).astype(mybir.dt.np(BF16))
    return {"xh": xh, "xl": xl, "cosq": cosq, "sinq": sinq, "tri": trim}


def host_inputs(common, Wq, Wk, Wv, Wo, k_scale, v_scale, core):
    wq_c = Wq[:, core * 512:(core + 1) * 512].astype(np.float32)
    wk_c = Wk[:, core * 256:(core + 1) * 256].astype(np.float32)
    wv_c = Wv[:, core * 256:(core + 1) * 256].astype(np.float32)
    wkv_c = np.concatenate([wk_c, wv_c], axis=1)
    wo_c = Wo[core * 512:(core + 1) * 512, :].astype(np.float32)
    wqh, wql = _fp8_split(wq_c, S_W)
    wkvh, wkvl = _fp8_split(wkv_c, S_W)
    woh, wol = _fp8_split(wo_c, S_W)

    ksc = np.empty((128, 2), dtype=np.float32)
    ksc[:, 0] = (1.0 + k_scale[0:128]) * (1.0 / S_K) * S_XW
    ksc[:, 1] = (1.0 + k_scale[128:256]) * (1.0 / S_K) * S_XW
    vsc = np.broadcast_to(
        ((1.0 + v_scale) * (1.0 / S_V) * S_XW).astype(mybir.dt.np(BF16)),
        (128, 256)).copy()

    out = {"wqh": wqh, "wql": wql, "wkvh": wkvh, "wkvl": wkvl,
           "woh": woh, "wol": wol, "ksc": ksc, "vsc": vsc}
    out.update(common)
    return out


def kernel(x, Wq, Wk, Wv, Wo, k_scale, v_scale, mask, position):
    from concourse.bass_utils import run_bass_kernel_spmd
    b, s, d = x.shape
    nc = build_bass(b=b, s=s, d=d)
    common = host_common(x, position, b=b, s=s, d=d)
    in_maps = [
        host_inputs(common, Wq, Wk, Wv, Wo, k_scale, v_scale, core)
        for core in range(N_CORES)
    ]
    res = run_bass_kernel_spmd(nc, in_maps, list(range(N_CORES)))
    out = None
    for r in res.results:
        yc = r["y"].astype(np.float32)
        out = yc if out is None else out + yc
    return (out * Y_SCALE).reshape(b, s, d).astype(np.float32)
